# revision 26
# baseline (speedup 1.0000x reference)
"""Attention pooling kernel for Trainium2 (Bass/Tile), SPMD over 8 NeuronCores.

Reference computation (per batch b):
    scores[t] = x[b,t,:] @ q / sqrt(D)  (+ mask)
    attn      = softmax(scores)            # over t
    out[b,:]  = sum_t attn[t] * x[b,t,:]

Strategy: data-parallel over batch (4 batches per core). One pass over x
(read once from HBM — the hard floor: 64 MiB/core at ~350 GB/s ≈ 190 us):
  - x[b] viewed as [128 partitions, 64 cols, 512] with t = p*64 + n,
    streamed in [128, CHUNK, 512] chunks. In bf16 mode the SWDGE DMA casts
    fp32->bf16 in flight (halves SBUF traffic; rel err ~2e-3 vs 2e-2 gate).
  - scores: DVE fused scalar_tensor_tensor w/ accum_out — measured 683 ns
    per [128,512] tile (same fp32/bf16; no 2x mode for STT; GpSimd/Scalar
    alternatives measured 1614/984 ns and GpSimd traffic slows DVE).
  - masking: exp on ScalarE of raw scores, then exp *= mask (0/1 float) on
    GpSimd — identical to the -1e9-bias softmax (exp(s)*0 == 0). Scores
    are O(0.5) so no max-subtraction needed.
  - pooled accumulation on PE: psum[1,512] += exp_col.T @ x_tile over the
    64 tiles of a batch. Z = sum(exp) via ones-matmul. out = acc * (1/Z).
  - ENGINE-QUEUE DISCIPLINE (engine queues are strict FIFO in emission
    order): the x-stream DGE ops live on the GpSimd queue, so nothing may
    be emitted on GpSimd that waits on a long dependency chain before the
    next DGE. All post-score work for group g (mask-mult on GpSimd,
    matmuls on PE, per-batch epilogue) is emitted AFTER group g+1's
    DMA+scores+exp — by then exp(g) has executed, so the GpSimd queue
    never head-of-line-blocks the stream. (Violating this measured 35-40
    us of stream stalls at batch boundaries.)
  - the globally-last chunk is split into FINE-tile groups so the final
    score/matmul work overlaps its own DMA tail instead of serializing
    after the stream (tail was 27 us with a monolithic last chunk).
"""

import os

import numpy as np

import bass_rust as _br
import concourse.bass as bass
import concourse.tile as tile
from concourse import mybir
from concourse.bass_utils import run_bass_kernel_spmd

B, T, D = 32, 8192, 512
N_CORES = 8
BC = B // N_CORES  # batches per core
P = 128  # SBUF partitions
NCOL = T // P  # 64 tiles (columns) per batch
CHUNK = int(os.environ.get("AP_CHUNK", "8"))  # tiles per DMA chunk
NCHUNK = NCOL // CHUNK
FINE = int(os.environ.get("AP_FINE", "4"))  # tiles per group in last chunk
SCALE = 1.0 / float(np.sqrt(np.float32(D)))

F32 = mybir.dt.float32
I32 = mybir.dt.int32

MM_DTYPE = os.environ.get("AP_MM_DTYPE", "float32r")
XQ_ALT = os.environ.get("AP_XQ_ALT", "1") == "1"
XBUFS = int(os.environ.get("AP_XBUFS", "10"))
XT_DT = {
    "float32": mybir.dt.float32,
    "float32r": mybir.dt.float32r,
    "bfloat16": mybir.dt.bfloat16,
}[MM_DTYPE]


def _split_multi_waits(nc):
    """The walrus build in this container accepts only one sync-wait command
    per instruction; hoist extra waits onto standalone EventSemaphore
    instructions placed just before (same engine, program order preserved)."""
    for f in nc.m.functions:
        for b in f.blocks:
            insts = b.instructions
            new = []
            changed = False
            for inst in insts:
                si = inst.sync_info
                if si is not None and len(si.on_wait) > 1:
                    waits = list(si.on_wait)
                    for w in waits[:-1]:
                        ies = mybir.InstEventSemaphore(
                            name=f"I-waitsplit-{nc.next_id()}", ins=[], outs=[]
                        )
                        ies.engine = inst.engine
                        ies.sync_info = _br.SyncInfo(on_wait=[w], on_update=[])
                        new.append(ies)
                    inst.sync_info = _br.SyncInfo(
                        on_wait=[waits[-1]], on_update=list(si.on_update)
                    )
                    changed = True
                new.append(inst)
            if changed:
                b.instructions = new


def _build_bass():
    nc = bass.Bass(
        "TRN2", target_bir_lowering=False, debug=False, num_devices=N_CORES
    )
    x_dram_dt = mybir.dt.float32r if MM_DTYPE == "float32r" else F32
    x = nc.dram_tensor("x", [BC, T, D], x_dram_dt, kind="ExternalInput").ap()
    mask = nc.dram_tensor("mask", [BC, T], I32, kind="ExternalInput").ap()
    q = nc.dram_tensor("pool_query", [1, 1, D], F32, kind="ExternalInput").ap()
    out = nc.dram_tensor("out", [BC, D], F32, kind="ExternalOutput").ap()

    # t = p * NCOL + n  (partition-major): per-partition rows contiguous.
    xv = x.rearrange("b (p n) d -> b p n d", p=P)
    mall = mask.rearrange("b (p n) -> p b n", p=P)  # [128, BC, NCOL]

    cast_dma = XT_DT != x_dram_dt

    with tile.TileContext(nc) as tc:
        with (
            tc.tile_pool(name="const", bufs=1) as const_pool,
            tc.tile_pool(name="xp", bufs=XBUFS) as xpool,
            tc.tile_pool(name="fxp", bufs=CHUNK // FINE) as fxpool,
            tc.tile_pool(name="dvp", bufs=2) as dvprod,
            tc.tile_pool(name="sp", bufs=2) as spool,
            tc.tile_pool(name="exp", bufs=2) as xppool,
            tc.tile_pool(name="cs", bufs=2) as cspool,
            tc.tile_pool(name="ep", bufs=2) as epool,
            tc.tile_pool(name="pacc", bufs=2, space="PSUM") as pacc,
            tc.tile_pool(name="pz", bufs=2, space="PSUM") as pz,
        ):
            # one-time constants on the idle scalar HWDGE ring / DVE
            small_dma = nc.scalar if cast_dma else nc.gpsimd
            q_bcast = const_pool.tile([P, D], F32)
            q_src = bass.AP(tensor=q.tensor, offset=q.offset, ap=[[0, P], [1, D]])
            small_dma.dma_start(out=q_bcast, in_=q_src)

            ones_col = const_pool.tile([P, 1], F32)
            nc.vector.memset(ones_col, 1.0)

            q_x = const_pool.tile([P, D], XT_DT)
            nc.vector.tensor_copy(out=q_x, in_=q_bcast)

            m_i32 = const_pool.tile([P, BC * NCOL], I32)
            small_dma.dma_start(out=m_i32, in_=mall)
            m_f = const_pool.tile([P, BC * NCOL], F32)
            mask_eng = nc.vector if cast_dma else nc.gpsimd
            mask_eng.tensor_copy(out=m_f, in_=m_i32)

            # global group plan: (batch, start col, width)
            groups = []
            for b in range(BC):
                for c in range(NCHUNK):
                    n0 = c * CHUNK
                    if b == BC - 1 and c == NCHUNK - 1:
                        groups += [
                            (b, n0 + k * FINE, FINE)
                            for k in range(CHUNK // FINE)
                        ]
                    else:
                        groups.append((b, n0, CHUNK))

            # per-batch state tiles
            state = {}

            def batch_state(b):
                if b not in state:
                    state[b] = dict(
                        s_all=spool.tile([P, NCOL], F32, name="s_all"),
                        exp_all=xppool.tile([P, NCOL], XT_DT, name="exp_all"),
                        acc=pacc.tile([1, D], F32, name="acc"),
                        z=pz.tile([1, 1], F32, name="z"),
                    )
                return state[b]

            xts = [None] * len(groups)

            def emit_front(gi):
                """DMA + scores + exp for group gi."""
                b, n0, w = groups[gi]
                st = batch_state(b)
                pool = fxpool if w != CHUNK else xpool
                xt = pool.tile([P, w, D], XT_DT)
                xts[gi] = xt
                if cast_dma:
                    xdma = nc.gpsimd
                elif XQ_ALT:
                    xdma = nc.sync if gi % 2 == 0 else nc.scalar
                else:
                    xdma = nc.sync
                xdma.dma_start(out=xt, in_=xv[b, :, n0 : n0 + w, :])
                for j in range(w):
                    n = n0 + j
                    prod = dvprod.tile([P, D], XT_DT)
                    nc.vector.scalar_tensor_tensor(
                        out=prod,
                        in0=xt[:, j, :],
                        scalar=SCALE,
                        in1=q_x,
                        op0=mybir.AluOpType.mult,
                        op1=mybir.AluOpType.mult,
                        accum_out=st["s_all"][:, n : n + 1],
                    )
                cs = slice(n0, n0 + w)
                nc.scalar.activation(
                    out=st["exp_all"][:, cs],
                    in_=st["s_all"][:, cs],
                    func=mybir.ActivationFunctionType.Exp,
                )

            def emit_back(gi):
                """mask-mult + matmuls for group gi; epilogue at batch end.
                Called one group late so exp(gi) has already executed and
                the GpSimd mask op never blocks the next DGE."""
                b, n0, w = groups[gi]
                st = batch_state(b)
                xt = xts[gi]
                cs = slice(n0, n0 + w)
                mask_eng.tensor_tensor(
                    out=st["exp_all"][:, cs],
                    in0=st["exp_all"][:, cs],
                    in1=m_f[:, b * NCOL + n0 : b * NCOL + n0 + w],
                    op=mybir.AluOpType.mult,
                )
                for j in range(w):
                    n = n0 + j
                    nc.tensor.matmul(
                        st["acc"],
                        lhsT=st["exp_all"][:, n : n + 1],
                        rhs=xt[:, j, :],
                        start=(n == 0),
                        stop=(n == NCOL - 1),
                    )
                if n0 + w == NCOL:  # batch complete
                    colsum = cspool.tile([P, 1], F32)
                    nc.vector.reduce_sum(
                        colsum, st["exp_all"], axis=mybir.AxisListType.X
                    )
                    nc.tensor.matmul(
                        st["z"], lhsT=colsum, rhs=ones_col, start=True, stop=True
                    )
                    zrec = epool.tile([1, 1], F32)
                    nc.vector.reciprocal(zrec, st["z"])
                    out_row = epool.tile([1, D], F32)
                    nc.vector.tensor_scalar_mul(
                        out=out_row, in0=st["acc"], scalar1=zrec
                    )
                    odma = nc.sync if cast_dma else nc.gpsimd
                    odma.dma_start(out=out[b : b + 1, :], in_=out_row)

            for gi in range(len(groups)):
                emit_front(gi)
                if gi >= 1:
                    emit_back(gi - 1)
            emit_back(len(groups) - 1)

    _split_multi_waits(nc)
    return nc


def _run(x, mask, pool_query, trace=False):
    x = np.ascontiguousarray(np.asarray(x, dtype=np.float32))
    mask = np.ascontiguousarray(np.asarray(mask, dtype=np.int32))
    pool_query = np.ascontiguousarray(np.asarray(pool_query, dtype=np.float32))
    assert x.shape == (B, T, D) and mask.shape == (B, T)

    nc = _build_bass()
    in_maps = []
    for c in range(N_CORES):
        lo, hi = c * BC, (c + 1) * BC
        in_maps.append(
            {
                "x": np.ascontiguousarray(x[lo:hi]),
                "mask": np.ascontiguousarray(mask[lo:hi]),
                "pool_query": pool_query,
            }
        )
    res = run_bass_kernel_spmd(
        nc, in_maps, core_ids=list(range(N_CORES)), trace=trace
    )
    out = np.concatenate([r["out"] for r in res.results], axis=0)
    return out, res


def kernel(x, mask, pool_query):
    out, _ = _run(x, mask, pool_query)
    return out


# revision 27
# speedup vs baseline: 1.0409x; 1.0409x over previous
"""Attention pooling kernel for Trainium2 (Bass/Tile), SPMD over 8 NeuronCores.

Reference computation (per batch b):
    scores[t] = x[b,t,:] @ q / sqrt(D)  (+ mask)
    attn      = softmax(scores)            # over t
    out[b,:]  = sum_t attn[t] * x[b,t,:]

Strategy: data-parallel over batch (4 batches per core). One pass over x
(read once from HBM — the hard floor: 64 MiB/core at ~350 GB/s ≈ 190 us):
  - x[b] viewed as [128 partitions, 64 cols, 512] with t = p*64 + n,
    streamed in [128, CHUNK, 512] chunks. In bf16 mode the SWDGE DMA casts
    fp32->bf16 in flight (halves SBUF traffic; rel err ~2e-3 vs 2e-2 gate).
  - scores: DVE fused scalar_tensor_tensor w/ accum_out — measured 683 ns
    per [128,512] tile (same fp32/bf16; no 2x mode for STT; GpSimd/Scalar
    alternatives measured 1614/984 ns and GpSimd traffic slows DVE).
  - masking: exp on ScalarE of raw scores, then exp *= mask (0/1 float) on
    GpSimd — identical to the -1e9-bias softmax (exp(s)*0 == 0). Scores
    are O(0.5) so no max-subtraction needed.
  - pooled accumulation on PE: psum[1,512] += exp_col.T @ x_tile over the
    64 tiles of a batch. Z = sum(exp) via ones-matmul. out = acc * (1/Z).
  - ENGINE-QUEUE DISCIPLINE (engine queues are strict FIFO in emission
    order): the x-stream DGE ops live on the GpSimd queue, so nothing may
    be emitted on GpSimd that waits on a long dependency chain before the
    next DGE. All post-score work for group g (mask-mult on GpSimd,
    matmuls on PE, per-batch epilogue) is emitted AFTER group g+1's
    DMA+scores+exp — by then exp(g) has executed, so the GpSimd queue
    never head-of-line-blocks the stream. (Violating this measured 35-40
    us of stream stalls at batch boundaries.)
  - the globally-last chunk is split into FINE-tile groups so the final
    score/matmul work overlaps its own DMA tail instead of serializing
    after the stream (tail was 27 us with a monolithic last chunk).
"""

import os

import numpy as np

import bass_rust as _br
import concourse.bass as bass
import concourse.tile as tile
from concourse import mybir
from concourse.bass_utils import run_bass_kernel_spmd

B, T, D = 32, 8192, 512
N_CORES = 8
BC = B // N_CORES  # batches per core
P = 128  # SBUF partitions
NCOL = T // P  # 64 tiles (columns) per batch
CHUNK = int(os.environ.get("AP_CHUNK", "8"))  # tiles per DMA chunk
NCHUNK = NCOL // CHUNK
FINE = int(os.environ.get("AP_FINE", "4"))  # tiles per group in last chunks
NFINE_CHUNKS = int(os.environ.get("AP_NFINE", "2"))  # how many trailing chunks go fine
SCALE = 1.0 / float(np.sqrt(np.float32(D)))

F32 = mybir.dt.float32
I32 = mybir.dt.int32

MM_DTYPE = os.environ.get("AP_MM_DTYPE", "float32r")
XQ_ALT = os.environ.get("AP_XQ_ALT", "0") == "1"
XBUFS = int(os.environ.get("AP_XBUFS", "10"))
XT_DT = {
    "float32": mybir.dt.float32,
    "float32r": mybir.dt.float32r,
    "bfloat16": mybir.dt.bfloat16,
}[MM_DTYPE]


def _split_multi_waits(nc):
    """The walrus build in this container accepts only one sync-wait command
    per instruction; hoist extra waits onto standalone EventSemaphore
    instructions placed just before (same engine, program order preserved)."""
    for f in nc.m.functions:
        for b in f.blocks:
            insts = b.instructions
            new = []
            changed = False
            for inst in insts:
                si = inst.sync_info
                if si is not None and len(si.on_wait) > 1:
                    waits = list(si.on_wait)
                    for w in waits[:-1]:
                        ies = mybir.InstEventSemaphore(
                            name=f"I-waitsplit-{nc.next_id()}", ins=[], outs=[]
                        )
                        ies.engine = inst.engine
                        ies.sync_info = _br.SyncInfo(on_wait=[w], on_update=[])
                        new.append(ies)
                    inst.sync_info = _br.SyncInfo(
                        on_wait=[waits[-1]], on_update=list(si.on_update)
                    )
                    changed = True
                new.append(inst)
            if changed:
                b.instructions = new


def _build_bass():
    nc = bass.Bass(
        "TRN2", target_bir_lowering=False, debug=False, num_devices=N_CORES
    )
    x_dram_dt = mybir.dt.float32r if MM_DTYPE == "float32r" else F32
    x = nc.dram_tensor("x", [BC, T, D], x_dram_dt, kind="ExternalInput").ap()
    mask = nc.dram_tensor("mask", [BC, T], I32, kind="ExternalInput").ap()
    q = nc.dram_tensor("pool_query", [1, 1, D], F32, kind="ExternalInput").ap()
    out = nc.dram_tensor("out", [BC, D], F32, kind="ExternalOutput").ap()

    # t = p * NCOL + n  (partition-major): per-partition rows contiguous.
    xv = x.rearrange("b (p n) d -> b p n d", p=P)
    mall = mask.rearrange("b (p n) -> p b n", p=P)  # [128, BC, NCOL]

    cast_dma = XT_DT != x_dram_dt

    with tile.TileContext(nc) as tc:
        with (
            tc.tile_pool(name="const", bufs=1) as const_pool,
            tc.tile_pool(name="xp", bufs=XBUFS) as xpool,
            tc.tile_pool(name="fxp", bufs=CHUNK // FINE) as fxpool,
            tc.tile_pool(name="dvp", bufs=2) as dvprod,
            tc.tile_pool(name="sp", bufs=2) as spool,
            tc.tile_pool(name="exp", bufs=2) as xppool,
            tc.tile_pool(name="cs", bufs=2) as cspool,
            tc.tile_pool(name="ep", bufs=2) as epool,
            tc.tile_pool(name="pacc", bufs=2, space="PSUM") as pacc,
            tc.tile_pool(name="pz", bufs=2, space="PSUM") as pz,
        ):
            # one-time constants on the idle scalar HWDGE ring / DVE
            small_dma = nc.scalar if cast_dma else nc.gpsimd
            q_bcast = const_pool.tile([P, D], F32)
            q_src = bass.AP(tensor=q.tensor, offset=q.offset, ap=[[0, P], [1, D]])
            small_dma.dma_start(out=q_bcast, in_=q_src)

            ones_col = const_pool.tile([P, 1], F32)
            nc.vector.memset(ones_col, 1.0)

            q_x = const_pool.tile([P, D], XT_DT)
            nc.vector.tensor_copy(out=q_x, in_=q_bcast)

            m_i32 = const_pool.tile([P, BC * NCOL], I32)
            small_dma.dma_start(out=m_i32, in_=mall)
            m_f = const_pool.tile([P, BC * NCOL], F32)
            mask_eng = nc.vector if cast_dma else nc.gpsimd
            mask_eng.tensor_copy(out=m_f, in_=m_i32)

            # global group plan: (batch, start col, width)
            groups = []
            for b in range(BC):
                for c in range(NCHUNK):
                    n0 = c * CHUNK
                    if b == BC - 1 and c >= NCHUNK - NFINE_CHUNKS:
                        groups += [
                            (b, n0 + k * FINE, FINE)
                            for k in range(CHUNK // FINE)
                        ]
                    else:
                        groups.append((b, n0, CHUNK))

            # per-batch state tiles
            state = {}

            def batch_state(b):
                if b not in state:
                    state[b] = dict(
                        s_all=spool.tile([P, NCOL], F32, name="s_all"),
                        exp_all=xppool.tile([P, NCOL], XT_DT, name="exp_all"),
                        acc=pacc.tile([1, D], F32, name="acc"),
                        z=pz.tile([1, 1], F32, name="z"),
                    )
                return state[b]

            xts = [None] * len(groups)

            def emit_front(gi):
                """DMA + scores + exp for group gi."""
                b, n0, w = groups[gi]
                st = batch_state(b)
                pool = fxpool if w != CHUNK else xpool
                xt = pool.tile([P, w, D], XT_DT)
                xts[gi] = xt
                if cast_dma:
                    xdma = nc.gpsimd
                elif XQ_ALT:
                    xdma = nc.sync if gi % 2 == 0 else nc.scalar
                else:
                    xdma = nc.sync
                xdma.dma_start(out=xt, in_=xv[b, :, n0 : n0 + w, :])
                for j in range(w):
                    n = n0 + j
                    prod = dvprod.tile([P, D], XT_DT)
                    nc.vector.scalar_tensor_tensor(
                        out=prod,
                        in0=xt[:, j, :],
                        scalar=SCALE,
                        in1=q_x,
                        op0=mybir.AluOpType.mult,
                        op1=mybir.AluOpType.mult,
                        accum_out=st["s_all"][:, n : n + 1],
                    )
                cs = slice(n0, n0 + w)
                nc.scalar.activation(
                    out=st["exp_all"][:, cs],
                    in_=st["s_all"][:, cs],
                    func=mybir.ActivationFunctionType.Exp,
                )

            def emit_back(gi):
                """mask-mult + matmuls for group gi; epilogue at batch end.
                Called one group late so exp(gi) has already executed and
                the GpSimd mask op never blocks the next DGE."""
                b, n0, w = groups[gi]
                st = batch_state(b)
                xt = xts[gi]
                cs = slice(n0, n0 + w)
                mask_eng.tensor_tensor(
                    out=st["exp_all"][:, cs],
                    in0=st["exp_all"][:, cs],
                    in1=m_f[:, b * NCOL + n0 : b * NCOL + n0 + w],
                    op=mybir.AluOpType.mult,
                )
                for j in range(w):
                    n = n0 + j
                    nc.tensor.matmul(
                        st["acc"],
                        lhsT=st["exp_all"][:, n : n + 1],
                        rhs=xt[:, j, :],
                        start=(n == 0),
                        stop=(n == NCOL - 1),
                    )
                if n0 + w == NCOL:  # batch complete
                    colsum = cspool.tile([P, 1], F32)
                    nc.vector.reduce_sum(
                        colsum, st["exp_all"], axis=mybir.AxisListType.X
                    )
                    nc.tensor.matmul(
                        st["z"], lhsT=colsum, rhs=ones_col, start=True, stop=True
                    )
                    zrec = epool.tile([1, 1], F32)
                    nc.vector.reciprocal(zrec, st["z"])
                    out_row = epool.tile([1, D], F32)
                    nc.vector.tensor_scalar_mul(
                        out=out_row, in0=st["acc"], scalar1=zrec
                    )
                    odma = nc.sync if cast_dma else nc.gpsimd
                    odma.dma_start(out=out[b : b + 1, :], in_=out_row)

            for gi in range(len(groups)):
                emit_front(gi)
                if gi >= 1:
                    emit_back(gi - 1)
            emit_back(len(groups) - 1)

    _split_multi_waits(nc)
    return nc


def _run(x, mask, pool_query, trace=False):
    x = np.ascontiguousarray(np.asarray(x, dtype=np.float32))
    mask = np.ascontiguousarray(np.asarray(mask, dtype=np.int32))
    pool_query = np.ascontiguousarray(np.asarray(pool_query, dtype=np.float32))
    assert x.shape == (B, T, D) and mask.shape == (B, T)

    nc = _build_bass()
    in_maps = []
    for c in range(N_CORES):
        lo, hi = c * BC, (c + 1) * BC
        in_maps.append(
            {
                "x": np.ascontiguousarray(x[lo:hi]),
                "mask": np.ascontiguousarray(mask[lo:hi]),
                "pool_query": pool_query,
            }
        )
    res = run_bass_kernel_spmd(
        nc, in_maps, core_ids=list(range(N_CORES)), trace=trace
    )
    out = np.concatenate([r["out"] for r in res.results], axis=0)
    return out, res


def kernel(x, mask, pool_query):
    out, _ = _run(x, mask, pool_query)
    return out
